# revision 16
# baseline (speedup 1.0000x reference)
"""DRAW-style read attention on Trainium2 — data-parallel over batch on 8 NeuronCores.

reference math (per batch element):
    params = h @ W.T + b                         [5]
    g_x = 64.5*(p0+1)-0.5 ; g_y likewise
    sigma2 = exp(p2) ; delta = (127/31)*exp(p3) ; gamma = exp(p4)
    mu_x[n] = g_x + (n-15.5)*delta ; mu_y likewise
    FX[n,h] = exp(-(h-mu_x[n])^2/(2 sigma2)) / (Z_n + 1e-8)    (Z_n = row sum)
    FY[m,w] likewise
    patch_i = FX @ img_i @ FY.T   for img in (x_c0..2, xhat_c0..2)
    out = gamma * flatten(patches)               [6144]

device layout per core (local batch B=32):
    images arrive pre-cast to bf16 in [quad, h, b4, c, w] layout so every DMA
    partition line is a 3KB contiguous DRAM run
    all affine transforms of the 5 raw params are folded into a 10-column
    host-side W'' (+bias as a padded K-chunk), so one f32 params matmul gives
    [g_x, g_y, s, s, gamma, d, d, ~0, ~0, ~0] directly after one exp over
    cols 2:10 (s = sqrt(1/(2 sigma2)), d = delta)
    mu = g + offs*delta comes from TWO accumulated 0/1-selection matmuls
    (E and offs-weighted E2), giving psum cols [mu_x, mu_y, s, s, gamma]
    filterbanks in [bn, hw] layout: ACT squares+exp, GPSIMD normalize (gamma
    folded into FY's scale), PE transpose to FXT/FYT [hw, bn] bf16
    main loop over pairs of b: At[w,n] = img[h,w].T @ FXT_b (12 matmuls/pair)
    then batched mm2: 4 matmuls/pair with At quads as 128/64-col stationary
    weights -> patch psum [128(ic,n), 32(m)]; outputs DMA'd in psum-native
    layout (384B lines, sync queue) and re-flattened on host
"""

import math

import numpy as np

import concourse.bass as bass  # noqa: F401  (import keeps bass registered)
import concourse.mybir as mybir
import concourse.tile as tile
from concourse import bacc
from concourse.bass_utils import run_bass_kernel_spmd
from concourse.masks import make_identity

F32 = mybir.dt.float32
BF16 = mybir.dt.bfloat16

NCORES = 8
B = 32          # per-core batch shard
C = 3
IMG = 128
N = 32
DH = 1024
U = 2 * C       # images per batch element: x channels 0..2 then x_hat channels 0..2
NT = (B * N) // 128   # quads: tiles over the flattened (b, n) axis
KC = DH // 128 + 1    # contraction chunks for params (last chunk carries bias)
NP = B // 2           # batch pairs
DELTA_NORM = (IMG - 1.0) / (N - 1.0)


def build_nc(finalize=True):
    nc = bacc.Bacc("TRN2", target_bir_lowering=False, debug=False, num_devices=NCORES)
    AFT = mybir.ActivationFunctionType
    ALU = mybir.AluOpType

    # images pre-cast + pre-laid-out on host: [quad, h, b4, c, w] bf16
    x4_d = nc.declare_dram_parameter("x4", [NT, 128, 4 * C * IMG], BF16, isOutput=False)
    xh4_d = nc.declare_dram_parameter("xh4", [NT, 128, 4 * C * IMG], BF16, isOutput=False)
    # combined 128-partition consts: h chunks [KC,B], W'' chunks [KC,10], grid
    hw_d = nc.declare_dram_parameter("hw", [128, KC * B + KC * 10 + IMG], F32,
                                     isOutput=False)
    # combined selection matrices E | E2 (offs-weighted)
    E_d = nc.declare_dram_parameter("E", [B, 2 * NT * 128], F32, isOutput=False)
    # psum-native output layout; host re-flattens (see unpack_out)
    out_d = nc.declare_dram_parameter("out", [NT, 128, 2, 3, N], F32, isOutput=True)

    with tile.TileContext(nc) as tc:
        with (
            tc.tile_pool(name="consts", bufs=1) as consts,
            tc.tile_pool(name="fb", bufs=3) as fb,
            tc.tile_pool(name="imgb_p", bufs=4) as imgb_p,
            tc.tile_pool(name="atb_p", bufs=3) as atb_p,
            tc.tile_pool(name="outs_p", bufs=3) as outs_p,
            tc.tile_pool(name="ps_pro", bufs=1, space="PSUM") as ps_pro,
            tc.tile_pool(name="ps_tr", bufs=2, space="PSUM") as ps_tr,
            tc.tile_pool(name="ps_at", bufs=3, space="PSUM") as ps_at,
            tc.tile_pool(name="ps_pt", bufs=2, space="PSUM") as ps_pt,
        ):
            # ---- tiny on-chip consts first so ACT's table load runs at t~0
            ident = consts.tile([128, 128], BF16)
            make_identity(nc, ident)
            zeros = consts.tile([128, 1], F32)
            nc.vector.memset(zeros, 0.0)
            prime_t = consts.tile([1, 1], F32)
            nc.scalar.activation(prime_t, zeros[:1], AFT.Exp, scale=-1.0,
                                 bias=zeros[:1])

            # ---- const DMAs: params deps on SP queue ahead of x; E on ACT
            hw_sb = consts.tile([128, KC * B + KC * 10 + IMG], F32)
            nc.sync.dma_start(out=hw_sb, in_=hw_d[:])
            hkb_sb = hw_sb[:, 0:KC * B].rearrange("p (k b) -> p k b", k=KC)
            wkb_sb = hw_sb[:, KC * B:KC * B + KC * 10].rearrange(
                "p (k j) -> p k j", k=KC)
            grid_sb = hw_sb[:, KC * B + KC * 10:]
            E_sb = consts.tile([B, 2, NT, 128], F32)
            nc.scalar.dma_start(out=E_sb, in_=E_d[:])

            # ---- image quad loads: x on SP, x_hat on ACT; 4 quads prefetch
            def emit_quad_dma(t):
                imgb = imgb_p.tile([128, 2, 4, C, IMG], BF16, tag="imgb",
                                   name=f"imgb{t}")
                nc.sync.dma_start(out=imgb[:, 0], in_=x4_d[t])
                nc.scalar.dma_start(out=imgb[:, 1], in_=xh4_d[t])
                return imgb

            quads = {t: emit_quad_dma(t) for t in range(4)}

            # ---- params: one 10-col f32 matmul chain (bias in chunk KC-1) ----
            # ps_par and ps_e share one PSUM bank (they're sequential)
            ps_pro_t = ps_pro.tile([128, NT * 5 + 10], F32)
            ps_par = ps_pro_t[0:B, NT * 5:NT * 5 + 10]
            for k in range(KC):
                nc.tensor.matmul(ps_par, hkb_sb[:, k, :], wkb_sb[:, k, :],
                                 start=(k == 0), stop=(k == KC - 1))
            # tp2 = [g_x, g_y, s, s, gamma, d, d, ~0, ~0, ~0]
            tp2 = consts.tile([B, 10], F32)
            nc.vector.tensor_copy(tp2[:, 0:2], ps_par[:, 0:2])
            nc.scalar.activation(tp2[:, 2:10], ps_par[:, 2:10], AFT.Exp,
                                 bias=zeros[:B])

            # ---- expand to (b,n) partitions: ps_e[:, t] = E_t.T@tp2[:,0:5]
            #      + E2_t.T@tp2[:,5:10] = [mu_x, mu_y, s, s, gamma] ----
            ps_e = ps_pro_t[:, 0:NT * 5].rearrange("p (t j) -> p t j", t=NT)
            for t in range(NT):
                nc.tensor.matmul(ps_e[:, t, :], E_sb[:, 0, t, :], tp2[:, 0:5],
                                 start=True, stop=False)
                nc.tensor.matmul(ps_e[:, t, :], E_sb[:, 1, t, :], tp2[:, 5:10],
                                 start=False, stop=True)
            # sg cols per tile: [s, s, gamma]; fbp: [-mu_x*s, -mu_y*s]
            # (two steps: a DVE op may read at most one PSUM input)
            sg = consts.tile([128, NT, 3], F32)
            nc.vector.tensor_copy(sg, ps_e[:, :, 2:5])
            fbp = consts.tile([128, NT, 2], F32)
            nc.vector.scalar_tensor_tensor(fbp, ps_e[:, :, 0:2], -1.0,
                                           sg[:, :, 0:2],
                                           op0=ALU.mult, op1=ALU.mult)

            # both filterbanks bf16 (matmuls run bf16); gamma folded into FY
            FXT = consts.tile([128, B * N], BF16)
            FYT = consts.tile([128, B * N], BF16)

            def fbank2(t):
                # sq = (s*grid - s*mu)^2 ; X and Y share one exp pass
                sq = fb.tile([128, 2, IMG], F32, tag="sq")
                nc.scalar.activation(sq[:, 0, :], grid_sb, AFT.Square,
                                     scale=sg[:, t, 0:1], bias=fbp[:, t, 0:1])
                nc.scalar.activation(sq[:, 1, :], grid_sb, AFT.Square,
                                     scale=sg[:, t, 1:2], bias=fbp[:, t, 1:2])
                # bf16 e_un: filterbank values end up bf16 anyway, and it
                # unlocks the DVE 2x read mode for the row-sum reduce
                e_un = fb.tile([128, 2, IMG], BF16, tag="e_un")
                nc.scalar.activation(e_un, sq, AFT.Exp, scale=-1.0, bias=zeros)
                Z2 = fb.tile([128, 2], F32, tag="Z2")
                nc.vector.tensor_reduce(Z2, e_un, axis=mybir.AxisListType.X,
                                        op=ALU.add)
                # the reference ADDS eps — for borderline off-grid rows Z is
                # itself ~1e-8, so a max-clamp is NOT equivalent
                nc.gpsimd.tensor_scalar_add(Z2, Z2, 1e-8)
                invZ2 = fb.tile([128, 2], F32, tag="invZ2")
                nc.vector.reciprocal_approx_fast(invZ2, Z2)
                # fold gamma into FY's normalizer (tiny [128,1] gpsimd op)
                nc.gpsimd.tensor_mul(invZ2[:, 1:2], invZ2[:, 1:2], sg[:, t, 2:3])
                for j, FT in ((0, FXT), (1, FYT)):
                    Fn = fb.tile([128, IMG], BF16, tag="Fn")
                    nc.gpsimd.tensor_scalar_mul(Fn, e_un[:, j, :],
                                                invZ2[:, j:j + 1])
                    ps_t = ps_tr.tile([128, 128], BF16, tag="ps_t")
                    nc.tensor.transpose(ps_t, Fn, ident)
                    nc.vector.tensor_copy(FT[:, t * 128:(t + 1) * 128], ps_t)

            # ---- main loop: pairs of batch elements; mm2 pipelined one pair
            # behind so the atb copy latency hides under the next mm1 ----
            def mm1(P, imgb, pp):
                ps_a = ps_at.tile([128, 2, U, N], F32, tag="ps_a")
                for b2 in range(2):
                    b = 2 * P + b2
                    for i in range(2):
                        for c in range(C):
                            nc.tensor.matmul(ps_a[:, b2, i * C + c, :],
                                             imgb[:, i, 2 * pp + b2, c, :],
                                             FXT[:, b * N:(b + 1) * N],
                                             start=True, stop=True)
                atb = atb_p.tile([128, 2, U, N], BF16, tag="atb")
                nc.vector.tensor_copy(atb, ps_a)
                return atb

            tile_ps = {}

            def mm2(P, atb):
                # At quads as 128/64-col stationary weights: 4 matmuls/pair.
                # psum partitions: j0/j1 -> (ic0..3, n) of b0/b1, j2 -> (b2,
                # ic4..5, n) via the 64-col tile at column offset 64.
                # both pairs of a quad share one psum tile so the epilogue is
                # a single copy + DMA per quad
                tq, pp = P // 2, P % 2
                if pp == 0:
                    tile_ps[tq] = ps_pt.tile([128, 2, 3, N], F32, tag="ps_o",
                                             name=f"pso{tq}")
                ps_p = tile_ps[tq]
                b0, b1 = 2 * P, 2 * P + 1
                fy0 = FYT[:, b0 * N:(b0 + 1) * N]
                fy1 = FYT[:, b1 * N:(b1 + 1) * N]
                nc.tensor.matmul(ps_p[:, pp, 0, :], atb[:, 0, 0:4, :], fy0,
                                 start=True, stop=True)
                nc.tensor.matmul(ps_p[:, pp, 1, :], atb[:, 1, 0:4, :], fy1,
                                 start=True, stop=True)
                nc.tensor.matmul(ps_p[0:64, pp, 2, :], atb[:, 0, 4:6, :], fy0,
                                 start=True, stop=True)
                nc.tensor.matmul(ps_p[64:128, pp, 2, :], atb[:, 1, 4:6, :], fy1,
                                 start=True, stop=True, tile_position=(0, 64))
                if pp == 1:
                    outs = outs_p.tile([128, 2, 3, N], F32, tag="outs")
                    nc.scalar.copy(outs, tile_ps.pop(tq))
                    nc.sync.dma_start(out=out_d[tq], in_=outs)

            prev = None
            fbank2(0)
            fbank2(1)
            for t in range(NT):
                # image DMAs four quads ahead, filterbanks two tiles ahead
                if t + 4 < NT:
                    quads[t + 4] = emit_quad_dma(t + 4)
                if t + 2 < NT:
                    fbank2(t + 2)
                imgb = quads.pop(t)
                for pp in range(2):
                    P = 2 * t + pp
                    atb = mm1(P, imgb, pp)
                    if prev is not None:
                        mm2(*prev)
                    prev = (P, atb)
            mm2(*prev)

    if finalize:
        nc.finalize()
    return nc


_CACHE = {}


def _get_nc():
    if "nc" not in _CACHE:
        _CACHE["nc"] = build_nc()
    return _CACHE["nc"]


def host_constants():
    # E: 0/1 selection expanding per-b scalars to (b,n) partitions;
    # E2 = offs-weighted E so mu = E.T@g + E2.T@delta in one psum accumulation
    E = np.zeros((B, 2, NT, 128), np.float32)
    for t in range(NT):
        for p in range(128):
            b = (t * 128 + p) // N
            E[b, 0, t, p] = 1.0
            E[b, 1, t, p] = (p % N) - (N / 2.0 - 0.5)
    grid = np.broadcast_to(np.arange(IMG, dtype=np.float32), (128, IMG))
    return E.reshape(B, 2 * NT * 128), np.ascontiguousarray(grid)


def _fold_params(W, b):
    """Fold every affine transform of the raw 5 params into a 10-col W'', b''.

    cols: [g_x, g_y, s_pre, s_pre, lg_gamma, lg_delta, lg_delta, z, z, z]
    where after exp of cols 2:10 the row becomes
    [g_x, g_y, s, s, gamma, delta, delta, ~0, ~0, ~0]
    """
    half = (IMG + 1) / 2.0
    W2 = np.zeros((10, DH), np.float32)
    b2 = np.zeros((10,), np.float32)
    W2[0] = half * W[0]
    b2[0] = half * (b[0] + 1.0) - 0.5
    W2[1] = half * W[1]
    b2[1] = half * (b[1] + 1.0) - 0.5
    # s = sqrt(1/(2*sigma2)) = exp(-p2/2 + 0.5*ln(0.5))
    W2[2] = W2[3] = -0.5 * W[2]
    b2[2] = b2[3] = -0.5 * b[2] + 0.5 * math.log(0.5)
    W2[4] = W[4]
    b2[4] = b[4]
    W2[5] = W2[6] = W[3]
    b2[5] = b2[6] = b[3] + math.log(DELTA_NORM)
    b2[7:10] = -30.0     # exp -> ~1e-13, cancels the offs-weighted E2 term
    return W2, b2


def make_in_maps(x, x_hat, h_dec_prev, W_read, b_read):
    import ml_dtypes
    bf16 = ml_dtypes.bfloat16
    x = np.asarray(x, np.float32)
    x_hat = np.asarray(x_hat, np.float32)
    h = np.asarray(h_dec_prev, np.float32)
    W2, b2 = _fold_params(np.asarray(W_read, np.float32),
                          np.asarray(b_read, np.float32))
    E, grid = host_constants()

    def quadlay(a):
        # [32, C, H, W] f32 -> [quad, h, b4, c, w] bf16, contiguous
        q = a.reshape(NT, 4, C, IMG, IMG).transpose(0, 3, 1, 2, 4)
        return np.ascontiguousarray(q.astype(bf16)).reshape(NT, 128, 4 * C * IMG)

    # params operands in SBUF [p, k, ...] layout with bias as chunk KC-1
    Wp = np.zeros((KC * 128, 10), np.float32)
    Wp[:DH] = W2.T
    Wp[DH] = b2
    wkb = Wp.reshape(KC, 128, 10).transpose(1, 0, 2).reshape(128, KC * 10)
    in_maps = []
    for i in range(NCORES):
        sl = slice(i * B, (i + 1) * B)
        hp = np.zeros((KC * 128, B), np.float32)
        hp[:DH] = h[sl].T
        hp[DH] = 1.0
        hkb = hp.reshape(KC, 128, B).transpose(1, 0, 2).reshape(128, KC * B)
        hw = np.ascontiguousarray(
            np.concatenate([hkb, wkb, grid], axis=1))
        in_maps.append({
            "x4": quadlay(x[sl]),
            "xh4": quadlay(x_hat[sl]),
            "hw": hw,
            "E": E,
        })
    return in_maps


def unpack_out(o):
    """Device out [NT, 128, 2, 3, N] f32 -> [B, U*N*N] flattened reference layout."""
    o = np.asarray(o, np.float32)
    o = o.reshape(NT, 128, 2, 3, N).transpose(0, 2, 1, 3, 4).reshape(NP, 128, 3, N)
    full = np.empty((B, U * N * N), np.float32)
    a = o.reshape(NP, 4, N, 3, N)[:, :, :, 0:2, :]      # [P, ic, n, b2, m]
    full[:, :4 * N * N] = a.transpose(0, 3, 1, 2, 4).reshape(B, 4 * N * N)
    bpart = o.reshape(NP, 2, 2, N, 3, N)[:, :, :, :, 2, :]   # [P, b2, ic2, n, m]
    full[:, 4 * N * N:] = bpart.reshape(B, 2 * N * N)
    return full


def _install_ntff_hook():
    """The container's antenv package lacks axon_hooks; provide it so
    run_bass_kernel_spmd(trace=True) can capture an NTFF profile."""
    import sys
    import types
    if "antenv.axon_hooks" in sys.modules:
        return
    try:
        from trn_agent_boot.trn_boot import _ntff_profile_via_ctypes
    except ImportError:
        return
    mod = types.ModuleType("antenv.axon_hooks")
    hook = [_ntff_profile_via_ctypes("/opt/axon/libaxon_pjrt.so")]
    mod.set_axon_ntff_profile_hook = lambda h: hook.__setitem__(0, h)
    mod.get_axon_ntff_profile_hook = lambda: hook[0]
    sys.modules["antenv.axon_hooks"] = mod
    try:
        import antenv
        antenv.axon_hooks = mod
    except ImportError:
        pass


def run(inputs, trace=False, **spmd_kwargs):
    """Run on the 8 NeuronCores; returns (out [256, 6144] f32, BassKernelResults)."""
    if trace:
        _install_ntff_hook()
    nc = _get_nc()
    in_maps = make_in_maps(**inputs)
    res = run_bass_kernel_spmd(nc, in_maps, core_ids=list(range(NCORES)),
                               trace=trace, **spmd_kwargs)
    out = np.concatenate([unpack_out(res.results[i]["out"])
                          for i in range(NCORES)], axis=0)
    return out, res


def kernel(x, x_hat, h_dec_prev, W_read, b_read):
    out, _ = run(dict(x=x, x_hat=x_hat, h_dec_prev=h_dec_prev,
                      W_read=W_read, b_read=b_read))
    return out
